# revision 16
# baseline (speedup 1.0000x reference)
"""BitMoEFFN Trainium2 kernel — expert-parallel over 8 NeuronCores.

Strategy (dense expert-parallel):
  - Core c owns expert c: computes BitFFN_c(xq) for ALL T=2048 tokens, scales
    rows by its router combine weight column, returns partial output;
    host sums the 8 partials (the unshard for expert parallelism).
  - Matmuls run on integer quantization codes (exact small ints) in fp8
    (gate/up: |codes|<=7) and bf16 (down: |codes|<=127), accumulated in fp32
    PSUM -> bit-exact integer arithmetic, scales applied after.
  - Top-k(0.55*F) magnitude masking uses a16 = fp16(h * 127/max|h|) for
    counting, masking AND code rounding consistently; per-token threshold via
    14-iteration bisection with single-op fused |a|>=t counting
    (tensor_scalar op0=abs_max op1=is_ge with accum_out).

Layout: tokens on partitions for quant/reductions; x^T/h^T for matmul
contraction via bf16 DMA-transpose round trips through DRAM.
"""

import numpy as np

B, S, H, F, E, K = 2, 1024, 1024, 4096, 8, 2
T = B * S
TOPK_RATIO = 0.55
KTOP = int(np.ceil(TOPK_RATIO * F))  # 2253
EPS = 1e-8
MAGIC = 12582912.0     # 1.5 * 2^23: fp32 RNE rounding via add/sub
MAGIC16 = 1536.0       # 1.5 * 2^10: fp16 RNE rounding via add/sub
NMT = T // 128         # 16 token tiles
GRP = 2                # token tiles per bisection group
BISECT_ITERS = 14
BISECT_HI = 16.0       # observed per-token thresholds in a-space: [1.2, 6.3]
WCH = 1024             # weight-conversion streaming chunk width

_cache = {}


def _build():
    from contextlib import ExitStack
    import concourse.bass as bass
    import concourse.bacc as bacc
    import concourse.mybir as mybir
    import concourse.tile as tile
    from concourse import bass_isa

    dt = mybir.dt
    Alu = mybir.AluOpType
    Act = mybir.ActivationFunctionType
    Ax = mybir.AxisListType
    ts = bass.ts

    nc = bacc.Bacc("TRN2", target_bir_lowering=False, debug=False,
                   num_devices=E)

    x_d = nc.dram_tensor("x", [T, H], dt.float32, kind="ExternalInput")
    xT_d = nc.dram_tensor("xT", [H, T], dt.float32, kind="ExternalInput")
    wgT_d = nc.dram_tensor("wgT", [H, F], dt.float32, kind="ExternalInput")
    wuT_d = nc.dram_tensor("wuT", [H, F], dt.float32, kind="ExternalInput")
    wdT_d = nc.dram_tensor("wdT", [F, H], dt.float32, kind="ExternalInput")
    wrT_d = nc.dram_tensor("wrT", [H, E], dt.float32, kind="ExternalInput")
    esel_d = nc.dram_tensor("esel", [128, E], dt.float32, kind="ExternalInput")
    yT_d = nc.dram_tensor("yT", [H, T], dt.float32, kind="ExternalOutput")

    xq_d = nc.dram_tensor("xq_s", [T, H], dt.bfloat16)
    hq_d = nc.dram_tensor("hq_s", [T, F], dt.bfloat16)
    gam_d = nc.dram_tensor("gam_s", [T], dt.float32)
    pr_d = {n: nc.dram_tensor(f"pr_{n}", [129], dt.float32)
            for n in ["wr", "wg", "wu", "wd"]}

    f32 = dt.float32
    f16 = dt.float16
    bf16 = dt.bfloat16
    f8 = dt.float8e4

    with tile.TileContext(nc) as tc, ExitStack() as ctx:
        const = ctx.enter_context(tc.tile_pool(name="const", bufs=1))
        colp = ctx.enter_context(tc.tile_pool(name="colp", bufs=1))
        smallp = ctx.enter_context(tc.tile_pool(name="smallp", bufs=4))
        psum = ctx.enter_context(tc.tile_pool(name="psum", bufs=8, space="PSUM"))
        xqTp = ctx.enter_context(tc.tile_pool(name="xqTp", bufs=1))

        # persistent columns
        sxv = colp.tile([128, NMT], f32)      # per-token max|x|/7
        mxv = colp.tile([128, NMT], f32)      # per-token max|h|
        comb = colp.tile([128, NMT], f32)     # this expert's combine weight
        esel_sb = const.tile([128, E], f32)
        nc.sync.dma_start(esel_sb[:], esel_d[:, :])

        def par_allreduce(col, op, key):
            # cross-partition reduce of [128,1] via DRAM round trip, then
            # broadcast the scalar back to all 128 partitions (0-stride read)
            scr = pr_d[key]
            nc.gpsimd.dma_start(bass.AP(scr, 1, [[1, 128], [1, 1]]), col)
            row = smallp.tile([1, 128], f32, tag="prow", name="prow")
            nc.gpsimd.dma_start(row[:], bass.AP(scr, 1, [[0, 1], [1, 128]]))
            red = smallp.tile([1, 1], f32, tag="pred", name="pred")
            nc.vector.tensor_reduce(red[:], row[:], axis=Ax.X, op=op)
            nc.gpsimd.dma_start(bass.AP(scr, 0, [[1, 1], [1, 1]]), red[:])
            o = smallp.tile([128, 1], f32, tag="par", name="par_o")
            nc.gpsimd.dma_start(o[:], bass.AP(scr, 0, [[0, 128], [1, 1]]))
            return o

        # ================= prep phase: router + xq + xqT =================
        with tc.tile_pool(name="prep", bufs=2) as prep:
            # --- router weights: global absmax int8 quant (values, fp32) ---
            wr_sb = const.tile([128, E * (H // 128)], f32)
            wr3 = wr_sb[:].rearrange("p (k e) -> p k e", e=E)
            nc.sync.dma_start(wr3, wrT_d.rearrange("(k p) e -> p k e", p=128))
            srt = smallp.tile([128, 1], f32, tag="par", name="srt")
            nc.vector.tensor_reduce(srt[:], wr3, axis=Ax.XY, op=Alu.max,
                                    apply_absolute_value=True)
            srm = par_allreduce(srt[:], Alu.max, 'wr')
            nc.vector.tensor_scalar(srm[:], srm[:], EPS, 1.0 / 127.0,
                                    Alu.max, Alu.mult)
            inv_sr = smallp.tile([128, 1], f32, tag="par", name="inv_sr")
            nc.vector.reciprocal(inv_sr[:], srm[:])
            wrq = const.tile([128, E * (H // 128)], f32)
            nc.vector.tensor_scalar(wrq[:], wr_sb[:], inv_sr[:, 0:1], MAGIC,
                                    Alu.mult, Alu.add)
            nc.vector.tensor_scalar(wrq[:], wrq[:], MAGIC, 127.0,
                                    Alu.subtract, Alu.min)
            nc.vector.tensor_scalar(wrq[:], wrq[:], -127.0, srm[:, 0:1],
                                    Alu.max, Alu.mult)
            wrq3 = wrq[:].rearrange("p (k e) -> p k e", e=E)

            # --- router logits (fp32 matmul, tokens on partitions) ---
            Lall = colp.tile([128, NMT * E], f32)
            L3 = Lall[:].rearrange("p (m e) -> p m e", e=E)
            for m in range(NMT):
                pl = psum.tile([128, 512], f32, tag="mm", name=f"pl{m}")
                for kk in range(H // 128):
                    xt_t = prep.tile([128, 128], f32, tag="xrt", name="xrt")
                    nc.sync.dma_start(xt_t[:], xT_d[ts(kk, 128), ts(m, 128)])
                    nc.tensor.matmul(pl[:, 0:E], xt_t[:], wrq3[:, kk, :],
                                     start=(kk == 0), stop=(kk == H // 128 - 1))
                nc.scalar.copy(Lall[:, m * E:(m + 1) * E], pl[:, 0:E])

            # --- top-2-of-8 gating, normalized; this expert's column ---
            m1 = colp.tile([128, NMT], f32)
            nc.vector.tensor_reduce(m1[:], L3, axis=Ax.X, op=Alu.max)
            dL = colp.tile([128, NMT * E], f32)
            d3 = dL[:].rearrange("p (m e) -> p m e", e=E)
            nc.vector.tensor_tensor(
                d3, L3, m1[:, :, None].to_broadcast((128, NMT, E)), Alu.subtract)
            e1 = colp.tile([128, NMT * E], f32)
            e13 = e1[:].rearrange("p (m e) -> p m e", e=E)
            nc.vector.tensor_scalar(e13, d3, 0.0, None, Alu.is_ge)
            nc.vector.scalar_tensor_tensor(e13, e13, -1e30, d3, Alu.mult, Alu.add)
            m2d = colp.tile([128, NMT], f32)
            nc.vector.tensor_reduce(m2d[:], e13, axis=Ax.X, op=Alu.max)
            lc = colp.tile([128, NMT * E], f32)
            lc3 = lc[:].rearrange("p (m e) -> p m e", e=E)
            nc.vector.tensor_tensor(
                lc3, L3, esel_sb[:, None, :].to_broadcast((128, NMT, E)), Alu.mult)
            lcr = colp.tile([128, NMT], f32)
            nc.vector.tensor_reduce(lcr[:], lc3, axis=Ax.X, op=Alu.add)
            lcd = colp.tile([128, NMT], f32)
            nc.vector.tensor_tensor(lcd[:], lcr[:], m1[:], Alu.subtract)
            sel = colp.tile([128, NMT], f32)
            nc.vector.tensor_tensor(sel[:], lcd[:], m2d[:], Alu.is_ge)
            elc = colp.tile([128, NMT], f32)
            nc.scalar.activation(elc[:], lcd[:], Act.Exp)
            em2 = colp.tile([128, NMT], f32)
            nc.scalar.activation(em2[:], m2d[:], Act.Exp)
            nc.vector.tensor_scalar(em2[:], em2[:], 1.0, None, Alu.add)
            rden = colp.tile([128, NMT], f32)
            nc.vector.reciprocal(rden[:], em2[:])
            nc.vector.tensor_tensor(comb[:], elc[:], rden[:], Alu.mult)
            nc.vector.tensor_tensor(comb[:], comb[:], sel[:], Alu.mult)

            # --- int4 activation quant: xq codes -> DRAM bf16 ---
            for m in range(NMT):
                xt = prep.tile([128, H], f32, tag="xq_in", name="xq_in")
                nc.sync.dma_start(xt[:], x_d[ts(m, 128), :])
                mx = smallp.tile([128, 1], f32, tag="mx", name="mx_x")
                nc.vector.tensor_reduce(mx[:], xt[:], axis=Ax.X, op=Alu.max,
                                        apply_absolute_value=True)
                nc.vector.tensor_scalar(mx[:], mx[:], EPS, 1.0 / 7.0,
                                        Alu.max, Alu.mult)
                nc.vector.tensor_copy(sxv[:, m:m + 1], mx[:])
                inv = smallp.tile([128, 1], f32, tag="mx", name="inv_x")
                nc.vector.reciprocal(inv[:], mx[:])
                nc.vector.tensor_scalar(xt[:], xt[:], inv[:, 0:1], MAGIC,
                                        Alu.mult, Alu.add)
                nc.vector.tensor_scalar(xt[:], xt[:], MAGIC, 7.0,
                                        Alu.subtract, Alu.min)
                cb = prep.tile([128, H], bf16, tag="xq_b", name="xq_b")
                nc.vector.tensor_scalar(cb[:], xt[:], -7.0, None, Alu.max)
                nc.gpsimd.dma_start(xq_d[ts(m, 128), :], cb[:])

            # --- transpose xq via DRAM -> fp8 resident [H,T] strips ---
            xqT = []
            for kk in range(H // 128):
                tb = prep.tile([128, T], bf16, tag="xqT_b", name="xqT_b")
                nc.sync.dma_start_transpose(tb[:], xq_d[:, ts(kk, 128)])
                t8 = xqTp.tile([128, T], f8, tag=f"xqT{kk}", name=f"xqT{kk}")
                nc.vector.tensor_copy(t8[:], tb[:])
                xqT.append(t8)

        # ================= weight scales (mean |w|) =================
        def mean_scale(wmp, src_d, ntile, width, key):
            wch = min(WCH, width)
            nch = width // wch
            acc = smallp.tile([128, ntile * nch], f32, tag="wacc",
                              name=f"acc_{src_d.name}")
            for kk in range(ntile):
                for ch in range(nch):
                    wt = wmp.tile([128, wch], f32, tag="w_in", name="w_in")
                    nc.sync.dma_start(
                        wt[:], src_d[ts(kk, 128), ts(ch, wch)])
                    nc.vector.tensor_reduce(acc[:, kk * nch + ch:kk * nch + ch + 1],
                                            wt[:], axis=Ax.X, op=Alu.add,
                                            apply_absolute_value=True)
            tot = smallp.tile([128, 1], f32, tag="par", name="tot")
            nc.vector.tensor_reduce(tot[:], acc[:], axis=Ax.X, op=Alu.add)
            s = par_allreduce(tot[:], Alu.add, key)
            nc.vector.tensor_scalar(s[:], s[:], 1.0 / (ntile * 128 * width), None,
                                    Alu.mult)
            nc.vector.tensor_scalar(s[:], s[:], EPS, None, Alu.max)
            inv = smallp.tile([128, 1], f32, tag="par", name="w_inv")
            nc.vector.reciprocal(inv[:], s[:])
            return s, inv

        with tc.tile_pool(name="wmean", bufs=2) as wmp:
            s_wg, inv_wg = mean_scale(wmp, wgT_d, H // 128, F, 'wg')
            s_wu, inv_wu = mean_scale(wmp, wuT_d, H // 128, F, 'wu')
            s_wd, inv_wd = mean_scale(wmp, wdT_d, F // 128, H, 'wd')

        def tern_tiles(wcp, src_d, inv, ntile, width, out_dtype, pool, tagp):
            wch = min(WCH, width)
            nch = width // wch
            outs = []
            for kk in range(ntile):
                o = pool.tile([128, width], out_dtype, tag=f"{tagp}{kk}",
                              name=f"{tagp}{kk}")
                for ch in range(nch):
                    wt = wcp.tile([128, wch], f32, tag="w_in", name="w_in")
                    nc.sync.dma_start(wt[:], src_d[ts(kk, 128), ts(ch, wch)])
                    nc.vector.tensor_scalar(wt[:], wt[:], inv[:, 0:1], MAGIC,
                                            Alu.mult, Alu.add)
                    nc.vector.tensor_scalar(wt[:], wt[:], MAGIC, 1.0,
                                            Alu.subtract, Alu.min)
                    nc.vector.tensor_scalar(o[:, ts(ch, wch)], wt[:], -1.0, None,
                                            Alu.max)
                outs.append(o)
            return outs

        # ================= gate/up + h + bisect + hq =================
        with tc.tile_pool(name="wgu", bufs=1) as wp, \
             tc.tile_pool(name="hpool", bufs=2) as hpool, \
             tc.tile_pool(name="aap", bufs=GRP + 1) as aap, \
             tc.tile_pool(name="rup", bufs=GRP) as rup, \
             tc.tile_pool(name="sgp", bufs=2) as sgp, \
             tc.tile_pool(name="junkp", bufs=1) as junkp, \
             tc.tile_pool(name="hqp", bufs=2) as hqp, \
             tc.tile_pool(name="bisp", bufs=1) as bisp:
            with tc.tile_pool(name="wconv", bufs=2) as wcp:
                wgq = tern_tiles(wcp, wgT_d, inv_wg, H // 128, F, f8, wp, "wg")
                wuq = tern_tiles(wcp, wuT_d, inv_wu, H // 128, F, f8, wp, "wu")

            # per-token scale products alpha = s_x*s_wg, beta = s_x*s_wu
            alv = colp.tile([128, NMT], f32)
            bev = colp.tile([128, NMT], f32)
            nc.vector.tensor_tensor(alv[:], sxv[:],
                                    s_wg[:, 0:1].to_broadcast((128, NMT)), Alu.mult)
            nc.vector.tensor_tensor(bev[:], sxv[:],
                                    s_wu[:, 0:1].to_broadcast((128, NMT)), Alu.mult)

            for g in range(NMT // GRP):
                a16s = []
                for mi in range(GRP):
                    m = g * GRP + mi
                    h_t = hpool.tile([128, F], f32, tag="h", name="h")
                    for half in range(2):
                        pg = [psum.tile([128, 512], f32, tag="mm", name=f"pg{j}")
                              for j in range(4)]
                        pu = [psum.tile([128, 512], f32, tag="mm", name=f"pu{j}")
                              for j in range(4)]
                        for kk in range(H // 128):
                            lhs = xqT[kk][:, ts(m, 128)]
                            st, sp = kk == 0, kk == H // 128 - 1
                            for j in range(4):
                                col = half * 2048 + j * 512
                                nc.tensor.matmul(pg[j][:], lhs,
                                                 wgq[kk][:, col:col + 512],
                                                 start=st, stop=sp)
                                nc.tensor.matmul(pu[j][:], lhs,
                                                 wuq[kk][:, col:col + 512],
                                                 start=st, stop=sp)
                        for j in range(4):
                            col = half * 2048 + j * 512
                            sg = sgp.tile([128, 512], f32, tag="sg", name="sg")
                            nc.scalar.activation(sg[:], pg[j][:], Act.Silu,
                                                 scale=alv[:, m:m + 1])
                            nc.vector.scalar_tensor_tensor(
                                h_t[:, col:col + 512], pu[j][:], bev[:, m:m + 1],
                                sg[:], Alu.mult, Alu.mult)
                    mx = smallp.tile([128, 1], f32, tag="mx", name="mx_h")
                    nc.vector.tensor_reduce(mx[:], h_t[:], axis=Ax.X, op=Alu.max,
                                            apply_absolute_value=True)
                    nc.vector.tensor_scalar(mx[:], mx[:], EPS, None, Alu.max)
                    nc.vector.tensor_copy(mxv[:, m:m + 1], mx[:])
                    inv = smallp.tile([128, 1], f32, tag="mx", name="inv_h")
                    nc.vector.reciprocal(inv[:], mx[:])
                    nc.vector.tensor_scalar(inv[:], inv[:], 127.0, None, Alu.mult)
                    rA = junkp.tile([128, F], f16, tag="junk", name="rA")
                    nc.vector.tensor_scalar(rA[:], h_t[:], inv[:, 0:1], None,
                                            Alu.mult)
                    aa16 = aap.tile([128, F], f16, tag="aa16", name="aa16")
                    nc.vector.tensor_scalar(
                        aa16[:].bitcast(dt.uint16), rA[:].bitcast(dt.uint16),
                        32767, None, Alu.bitwise_and)
                    rU = rup.tile([128, F], dt.int8, tag="rU", name="rU")
                    nc.vector.tensor_scalar(rU[:], rA[:], MAGIC16, MAGIC16,
                                            Alu.add, Alu.subtract)
                    a16s.append((aa16, rU))

                # bisect per-token threshold on |a16| counts (fp16-grid exact)
                lo = bisp.tile([128, GRP], f32, tag="lo", name="lo")
                hi = bisp.tile([128, GRP], f32, tag="hi", name="hi")
                mid = bisp.tile([128, GRP], f32, tag="mid", name="mid")
                cnt = bisp.tile([128, GRP], f32, tag="cnt", name="cnt")
                ge = bisp.tile([128, GRP], dt.int8, tag="ge", name="ge")
                nge = bisp.tile([128, GRP], dt.int8, tag="nge", name="nge")
                nc.vector.memset(lo[:], 0.0)
                nc.vector.memset(hi[:], BISECT_HI)
                for it in range(BISECT_ITERS):
                    nc.vector.tensor_tensor(mid[:], lo[:], hi[:], Alu.add)
                    nc.vector.tensor_scalar(mid[:], mid[:], 0.5, None, Alu.mult)
                    for mi in range(GRP):
                        junk = junkp.tile([128, F], f16, tag="junk", name="junk")
                        nc.vector.tensor_scalar(
                            junk[:], a16s[mi][0][:], mid[:, mi:mi + 1], None,
                            Alu.is_ge, Alu.add,
                            accum_out=cnt[:, mi:mi + 1])
                    nc.vector.tensor_scalar(ge[:], cnt[:], float(KTOP), None,
                                            Alu.is_ge)
                    nc.vector.copy_predicated(lo[:], ge[:], mid[:])
                    nc.vector.tensor_scalar(nge[:], ge[:], -1.0, 1.0,
                                            Alu.mult, Alu.add)
                    nc.vector.copy_predicated(hi[:], nge[:], mid[:])

                # mask + RNE-round codes (in-place on a16) + store hq bf16
                for mi in range(GRP):
                    m = g * GRP + mi
                    mk = junkp.tile([128, F], f16, tag="junk", name="mk")
                    nc.vector.tensor_scalar(mk[:], a16s[mi][0][:],
                                            lo[:, mi:mi + 1], None, Alu.is_ge)
                    hqb = hqp.tile([128, F], bf16, tag="hqb", name="hqb")
                    nc.vector.tensor_tensor(hqb[:], a16s[mi][1][:], mk[:],
                                            Alu.mult)
                    nc.gpsimd.dma_start(hq_d[ts(m, 128), :], hqb[:])

        # ============ combine scale gamma -> broadcast row ============
        gam = colp.tile([128, NMT], f32)
        nc.vector.tensor_tensor(gam[:], mxv[:],
                                s_wd[:, 0:1].to_broadcast((128, NMT)), Alu.mult)
        nc.vector.tensor_scalar(gam[:], gam[:], 1.0 / 127.0, None, Alu.mult)
        nc.vector.tensor_tensor(gam[:], gam[:], comb[:], Alu.mult)
        nc.gpsimd.dma_start(gam_d.rearrange("(m p) -> p m", p=128), gam[:])

        # ============ down matmul: yT[h,t] = wd_codes^T @ hq^T ============
        with tc.tile_pool(name="wd", bufs=1) as wdp, \
             tc.tile_pool(name="wconv2", bufs=2) as wcp2, \
             tc.tile_pool(name="strp", bufs=3) as strp, \
             tc.tile_pool(name="outp", bufs=3) as outp:
            gbc = wdp.tile([128, T], f32, tag="gbc", name="gbc")
            nc.sync.dma_start(gbc[:], bass.AP(gam_d, 0, [[0, 128], [1, T]]))
            wdq = tern_tiles(wcp2, wdT_d, inv_wd, F // 128, H, bf16, wdp, "wd")
            for tcb in range(4):
                py = [psum.tile([128, 512], f32, tag="mm", name=f"py{j}")
                      for j in range(8)]
                for kk in range(F // 128):
                    strip = strp.tile([128, 512], bf16, tag="strip", name="strip")
                    nc.sync.dma_start_transpose(
                        strip[:], hq_d[ts(tcb, 512), ts(kk, 128)])
                    st, sp = kk == 0, kk == F // 128 - 1
                    for hh in range(8):
                        nc.tensor.matmul(py[hh][:], wdq[kk][:, ts(hh, 128)],
                                         strip[:], start=st, stop=sp)
                for hh in range(8):
                    yt = outp.tile([128, 512], f32, tag="yt", name="yt")
                    nc.vector.tensor_tensor(yt[:], py[hh][:],
                                            gbc[:, ts(tcb, 512)], Alu.mult)
                    nc.gpsimd.dma_start(yT_d[ts(hh, 128), ts(tcb, 512)], yt[:])

    nc.compile()
    return nc


def kernel(x, w_gate, w_up, w_down, w_router):
    from concourse.bass_utils import run_bass_kernel_spmd

    if "nc" not in _cache:
        _cache["nc"] = _build()
    nc = _cache["nc"]

    x = np.asarray(x, np.float32)
    xf = np.ascontiguousarray(x.reshape(T, H))
    xT = np.ascontiguousarray(xf.T)
    wrT = np.ascontiguousarray(np.asarray(w_router, np.float32).T)
    in_maps = []
    for c in range(E):
        esel = np.zeros((128, E), np.float32)
        esel[:, c] = 1.0
        in_maps.append({
            "x": xf,
            "xT": xT,
            "wgT": np.ascontiguousarray(np.asarray(w_gate[c], np.float32).T),
            "wuT": np.ascontiguousarray(np.asarray(w_up[c], np.float32).T),
            "wdT": np.ascontiguousarray(np.asarray(w_down[c], np.float32).T),
            "wrT": wrT,
            "esel": esel,
        })
    res = run_bass_kernel_spmd(nc, in_maps, list(range(E)))
    out = np.zeros((H, T), np.float32)
    for c in range(E):
        out += res.results[c]["yT"]
    return np.ascontiguousarray(out.T).reshape(B, S, H).astype(np.float32)
